# revision 4
# baseline (speedup 1.0000x reference)
"""CrossBatchAttention Trainium2 kernel — 8-core tensor-parallel SPMD.

v3: head-parallel attention + BATCH-parallel out_proj/gate via AllToAll.

The expensive collectives of the tensor-parallel formulation (8MB OT
AllGather + gate ReduceScatter/AllGather chain, ~290us of CC) are replaced
by one small AllToAll (split in two for overlap): after attention, core c
keeps its 4 heads' O^T for batch shard r and sends [512, 256] fp8 to core
r. Each core then owns the FULL attention output for its 256-row batch
shard and runs out_proj + the whole gate MLP locally with full (fp8)
weights streamed into SBUF. Total CC traffic drops ~14x.

fp8 + MatmulPerfMode.DoubleRow everywhere the contraction depth allows.
Softmax: exp is shifted by a constant C (folded into the mask bias) so P
stays under fp8e4's 240 max; numerator and denominator share the same fp8
P so quantization largely cancels. The self-attention diagonal is zeroed
in PSUM before exp (leaves a negligible exp(-C) self-weight).

Per core c (of 8):
  phase 1: QT/KT bf16 [512,2048], V fp8 (4 local heads) from fp8 X^T and
           fp8 resident weights via DoubleRow, streamed in batch-quarters;
           g1x = gW1[:H]^T @ X^T[:, own 256 cols] (full gate hidden, own
           batch shard) from a per-core input slice.
  phase 2: per (head, batch-quarter): S^T = K^T@Q^T per j-tile (bf16),
           diag-zero in PSUM, Exp(scale*s + maskb - C) straight to fp8,
           denominator + O^T via DoubleRow over j-tile pairs, normalize,
           scatter O^T fp8 into A2A send buffers; AllToAll after head 1
           (heads 0-1) and after head 3 (heads 2-3).
  phase 3: otg [4096, 256] = gathered O^T for own shard; cross^T[:, own]
           = Wo_p^T @ otg (full hid, DoubleRow, Wo streamed in 8 chunks);
           g = gelu(g1x + gW1c^T @ cross8 + b1) fully local; gate =
           sigmoid(gW2^T g + b2); out^T = gate * cross^T  [4096, 256].
Host: concat 8 [4096,256] shards along batch, transpose, add X.
"""

import numpy as np
import ml_dtypes

import concourse.bass as bass
import concourse.mybir as mybir
import concourse.tile as tile
from concourse import bacc
from concourse import bass_utils

BF16 = mybir.dt.bfloat16
F32 = mybir.dt.float32
F8 = mybir.dt.float8e4
WO_SCALE = 64.0
EXP_SHIFT = 5.0          # exp(s - C): keeps P < 240 (fp8e4 max); cancels in softmax

B = 2048
HID = 4096
NH = 32
HD = 128
GH = 1024
NC_ = 8
HPC = NH // NC_          # heads per core = 4
HS = HID // NC_          # hid shard = 512 (attention head shard)
BS = B // NC_            # batch shard = 256 (out_proj/gate shard)
SCALE = 1.0 / float(np.sqrt(HD))

KT_TILES = HID // 128    # 32 k-tiles over the 4096 contraction
KP = KT_TILES // 2       # 16 DoubleRow k-pairs
JT = B // 128            # 16 j-tiles over keys
JP = JT // 2             # 8 DoubleRow j-pairs
IC = B // 512            # 4 i-chunks of 512 over batch (attention tiling)

DR = mybir.MatmulPerfMode.DoubleRow

GELU_FUNC = mybir.ActivationFunctionType.Gelu


def _build_program():
    nc = bacc.Bacc(
        "TRN2",
        target_bir_lowering=False,
        debug=False,
        enable_asserts=False,
        num_devices=NC_,
    )

    # ---- I/O declarations (per-core shapes) ----
    xt_d = nc.dram_tensor("xt8", [HID, B], F8, kind="ExternalInput").ap()
    xo_d = nc.dram_tensor("xo8", [HID, BS], F8, kind="ExternalInput").ap()
    wq_d = nc.dram_tensor("wq", [HID, HS], F8, kind="ExternalInput").ap()
    wk_d = nc.dram_tensor("wk", [HID, HS], F8, kind="ExternalInput").ap()
    wv_d = nc.dram_tensor("wv", [HID, HS], F8, kind="ExternalInput").ap()
    wo_d = nc.dram_tensor("wo", [HID, HID], F8, kind="ExternalInput").ap()
    gw1x_d = nc.dram_tensor("gw1x", [HID, GH], F8, kind="ExternalInput").ap()
    gw1c_d = nc.dram_tensor("gw1c", [HID, GH], F8, kind="ExternalInput").ap()
    gw2_d = nc.dram_tensor("gw2", [GH, HID], F8, kind="ExternalInput").ap()
    gb1_d = nc.dram_tensor("gb1", [128, GH // 128], F32,
                           kind="ExternalInput").ap()
    gb2_d = nc.dram_tensor("gb2", [128, KT_TILES], F32,
                           kind="ExternalInput").ap()
    maskb_d = nc.dram_tensor("maskb", [128, JT], F32, kind="ExternalInput").ap()
    diagm_d = nc.dram_tensor("diagm", [128, 128], F32, kind="ExternalInput").ap()
    out_d = nc.dram_tensor("out", [HID, BS], F32, kind="ExternalOutput").ap()

    groups = [list(range(NC_))]

    with tile.TileContext(nc) as tc:
        with (
            tc.tile_pool(name="persist", bufs=1) as persist,
            tc.tile_pool(name="psum", bufs=1, space="PSUM") as psum,
            tc.tile_pool(name="dram", bufs=1, space="DRAM") as dram,
        ):
            # ---------- persistent SBUF ----------
            g1x_sb = persist.tile([128, GH // 128, BS], F32)  # 1MB
            maskb_sb = persist.tile([128, JT], F32)
            diagm_sb = persist.tile([128, 128], F32)
            ones_sb = persist.tile([128, 2, 128], F8)
            gb1_sb = persist.tile([128, GH // 128], F32)
            gb2_sb = persist.tile([128, KT_TILES], F32)

            nc.sync.dma_start(out=maskb_sb, in_=maskb_d)
            nc.sync.dma_start(out=diagm_sb, in_=diagm_d)
            nc.sync.dma_start(out=gb1_sb, in_=gb1_d)
            nc.sync.dma_start(out=gb2_sb, in_=gb2_d)
            nc.vector.memset(ones_sb, 1.0)

            # ---------- DRAM buffers for the A2A ----------
            # src_a: heads 0-1, src_b: heads 2-3. Block r ([256,256] rows
            # r*256..) = OT_local[(h%2)*128+d, r*256:(r+1)*256].
            src_a = dram.tile([B, BS], F8, name="src_a")
            src_b = dram.tile([B, BS], F8, name="src_b")
            dst_a = dram.tile([B, BS], F8, name="dst_a")
            dst_b = dram.tile([B, BS], F8, name="dst_b")

            warm_i = dram.tile([NC_ * 16, 64], F8)
            warm_o = dram.tile([NC_ * 16, 64], F8)
            nc.gpsimd.collective_compute(
                "AllToAll", mybir.AluOpType.bypass, replica_groups=groups,
                ins=[warm_i.opt()], outs=[warm_o.opt()],
            )

            # =====================================================
            # Phase 1: projections, streamed in batch-quarters.
            # All weights fp8, resident; DoubleRow over k-pairs.
            # =====================================================
            with (
                tc.tile_pool(name="p12", bufs=1) as p12,
                tc.tile_pool(name="p1", bufs=1) as p1,
            ):
                qt_sb = p12.tile([128, HPC, B], BF16)     # [d, head, i] 2MB
                kt_sb = p12.tile([128, HPC, B], BF16)     # 2MB
                v_sb = p12.tile([128, JT, HS], F8)        # [j_in, jt, hd] 1MB

                wq_sb = p1.tile([128, KT_TILES, HS], F8, tag="wq", bufs=1)
                wk_sb = p1.tile([128, KT_TILES, HS], F8, tag="wk", bufs=1)
                wv_sb = p1.tile([128, KT_TILES, HS], F8, tag="wv", bufs=1)
                gw1x_sb = p1.tile([128, KT_TILES, GH], F8, tag="gw1x", bufs=1)
                xo_sb = p1.tile([128, KT_TILES, BS], F8, tag="xo", bufs=1)
                for wd, wsb in ((wq_d, wq_sb), (wk_d, wk_sb), (wv_d, wv_sb)):
                    for hh in range(4):
                        nc.sync.dma_start(
                            out=wsb[:, hh * 8:(hh + 1) * 8, :],
                            in_=wd[hh * 1024:(hh + 1) * 1024, :].rearrange(
                                "(t p) m -> p t m", p=128
                            ),
                        )
                nc.sync.dma_start(
                    out=xo_sb, in_=xo_d.rearrange("(t p) m -> p t m", p=128)
                )
                for hh in range(4):
                    nc.sync.dma_start(
                        out=gw1x_sb[:, hh * 8:(hh + 1) * 8, :],
                        in_=gw1x_d[hh * 1024:(hh + 1) * 1024, :].rearrange(
                            "(t p) m -> p t m", p=128
                        ),
                    )

                for q in range(IC):  # 4 quarters of 512 batch elems
                    isl = slice(q * 512, (q + 1) * 512)
                    xt_q = p1.tile([128, KT_TILES, 512], F8, tag="xt", bufs=2)
                    # chunked DMA so the first matmuls start early
                    for kk in range(4):
                        nc.sync.dma_start(
                            out=xt_q[:, kk * 8:(kk + 1) * 8, :],
                            in_=xt_d[kk * 1024:(kk + 1) * 1024, isl].rearrange(
                                "(t p) i -> p t i", p=128
                            ),
                        )

                    for wsb, dst in ((wq_sb, qt_sb), (wk_sb, kt_sb)):
                        for m in range(4):
                            msl = slice(m * 128, (m + 1) * 128)
                            ps = psum.tile([128, 512], F32, tag="mm", bufs=4,
                                           name="ps_pr")
                            for k in range(KP):
                                nc.tensor.matmul(
                                    ps,
                                    lhsT=wsb[:, 2 * k:2 * k + 2, msl],
                                    rhs=xt_q[:, 2 * k:2 * k + 2, :],
                                    start=(k == 0),
                                    stop=(k == KP - 1),
                                    perf_mode=DR,
                                )
                            nc.vector.tensor_copy(dst[:, m, isl], ps)
                    # V in natural [j, d] layout: lhsT = X^T tiles
                    for it in range(4):  # 4 i-tiles of 128 in this quarter
                        ps = psum.tile([128, 512], F32, tag="mm", bufs=4,
                                       name="ps_v")
                        for k in range(KP):
                            nc.tensor.matmul(
                                ps,
                                lhsT=xt_q[:, 2 * k:2 * k + 2,
                                          it * 128:(it + 1) * 128],
                                rhs=wv_sb[:, 2 * k:2 * k + 2, :],
                                start=(k == 0),
                                stop=(k == KP - 1),
                                perf_mode=DR,
                            )
                        nc.vector.tensor_copy(v_sb[:, q * 4 + it, :], ps)

                # gate W1 X-part: full gate hidden for the own batch shard
                for gm in range(GH // 128):
                    ps = psum.tile([128, BS], F32, tag="mm", bufs=4,
                                   name="ps_g1x")
                    for k in range(KP):
                        nc.tensor.matmul(
                            ps,
                            lhsT=gw1x_sb[:, 2 * k:2 * k + 2,
                                         gm * 128:(gm + 1) * 128],
                            rhs=xo_sb[:, 2 * k:2 * k + 2, :],
                            start=(k == 0),
                            stop=(k == KP - 1),
                            perf_mode=DR,
                        )
                    nc.vector.tensor_copy(g1x_sb[:, gm, :], ps)

                # =====================================================
                # Phase 2: attention per (head, batch-quarter)
                # =====================================================
                with tc.tile_pool(name="p2", bufs=1) as p2:
                    for h in range(HPC):
                        srcbuf = src_a if h < 2 else src_b
                        hh = h % 2
                        for q in range(IC):
                            qsl = slice(q * 512, (q + 1) * 512)
                            den_ps = psum.tile([128, 512], F32, tag="den",
                                               bufs=2)
                            ot_ps = psum.tile([128, 512], F32, tag="ot",
                                              bufs=2)
                            pt = p2.tile([128, JT, 512], F8, tag="pt", bufs=2)
                            for j in range(JT):
                                st = psum.tile([128, 512], F32, tag="mm",
                                               bufs=4, name="st")
                                nc.tensor.matmul(
                                    st,
                                    lhsT=kt_sb[:, h, j * 128:(j + 1) * 128],
                                    rhs=qt_sb[:, h, qsl],
                                    start=True,
                                    stop=True,
                                )
                                # zero the self-attn diagonal block in PSUM
                                if j // 4 == q:
                                    c0 = (j % 4) * 128
                                    nc.vector.tensor_mul(
                                        st[:, c0:c0 + 128],
                                        st[:, c0:c0 + 128],
                                        diagm_sb,
                                    )
                                nc.scalar.activation(
                                    pt[:, j, :],
                                    st,
                                    mybir.ActivationFunctionType.Exp,
                                    bias=maskb_sb[:, j:j + 1],
                                    scale=SCALE,
                                )
                            for j in range(JP):
                                nc.tensor.matmul(
                                    den_ps,
                                    lhsT=ones_sb,
                                    rhs=pt[:, 2 * j:2 * j + 2, :],
                                    start=(j == 0),
                                    stop=(j == JP - 1),
                                    perf_mode=DR,
                                )
                                nc.tensor.matmul(
                                    ot_ps,
                                    lhsT=v_sb[:, 2 * j:2 * j + 2,
                                              h * 128:(h + 1) * 128],
                                    rhs=pt[:, 2 * j:2 * j + 2, :],
                                    start=(j == 0),
                                    stop=(j == JP - 1),
                                    perf_mode=DR,
                                )
                            rec = p2.tile([128, 512], F32, tag="rec", bufs=2)
                            nc.vector.reciprocal_approx_fast(out=rec,
                                                             in_=den_ps)
                            otc = p2.tile([128, 512], F8, tag="otc", bufs=2)
                            nc.vector.tensor_mul(otc, ot_ps, rec)
                            # scatter into the A2A send buffer: batch shards
                            # r = 2q (first 256 cols) and 2q+1 (last 256)
                            for half in range(2):
                                r = 2 * q + half
                                nc.sync.dma_start(
                                    out=srcbuf[r * 256 + hh * 128:
                                               r * 256 + hh * 128 + 128, :],
                                    in_=otc[:, half * 256:half * 256 + 256],
                                )
                        if h == 1:
                            nc.gpsimd.collective_compute(
                                "AllToAll", mybir.AluOpType.bypass,
                                replica_groups=groups,
                                ins=[src_a.opt()], outs=[dst_a.opt()],
                            )
                        elif h == 3:
                            nc.gpsimd.collective_compute(
                                "AllToAll", mybir.AluOpType.bypass,
                                replica_groups=groups,
                                ins=[src_b.opt()], outs=[dst_b.opt()],
                            )

            # =====================================================
            # Phase 3: out_proj + gate MLP, fully local on the own
            # 256-row batch shard. Wo streamed in 8 chunks.
            # =====================================================
            with tc.tile_pool(name="p34", bufs=1) as p34:
                cacc = p34.tile([128, KT_TILES, BS], BF16, tag="cacc", bufs=1)
                cacc8 = p34.tile([128, KT_TILES, BS], F8, tag="cacc8", bufs=1)
                gw1c_sb = p34.tile([128, KT_TILES, GH], F8, tag="gw1c",
                                   bufs=1)
                for hh in range(4):
                    nc.sync.dma_start(
                        out=gw1c_sb[:, hh * 8:(hh + 1) * 8, :],
                        in_=gw1c_d[hh * 1024:(hh + 1) * 1024, :].rearrange(
                            "(t p) m -> p t m", p=128
                        ),
                    )
                gw2_sb = p34.tile([128, GH // 128, HID], F8, tag="gw2",
                                  bufs=1)
                for hh in range(2):
                    nc.sync.dma_start(
                        out=gw2_sb[:, hh * 4:(hh + 1) * 4, :],
                        in_=gw2_d[hh * 512:(hh + 1) * 512, :].rearrange(
                            "(t p) m -> p t m", p=128
                        ),
                    )
                # gathered O^T for the own batch shard: [128, pp, 16, 256]
                otg = p34.tile([128, 2, 16, BS], F8, tag="otg", bufs=1)
                nc.scalar.dma_start(
                    out=otg[:, 0],
                    in_=dst_a.rearrange("(t p) i -> p t i", p=128),
                )
                nc.scalar.dma_start(
                    out=otg[:, 1],
                    in_=dst_b.rearrange("(t p) i -> p t i", p=128),
                )

                # out_proj: cross^T[:, own shard], Wo streamed in 8 chunks
                for mc in range(8):
                    wo_mc = p34.tile([128, KT_TILES, 512], F8, tag="wo",
                                     bufs=4, name=f"wo{mc}")
                    nc.sync.dma_start(
                        out=wo_mc,
                        in_=wo_d[:, mc * 512:(mc + 1) * 512].rearrange(
                            "(t p) m -> p t m", p=128
                        ),
                    )
                    for m in range(4):
                        msl = slice(m * 128, (m + 1) * 128)
                        ps = psum.tile([128, BS], F32, tag="mm", bufs=4,
                                       name="ps_wo")
                        for pp in range(2):
                            for u in range(8):
                                nc.tensor.matmul(
                                    ps,
                                    lhsT=wo_mc[:, pp * 16 + 2 * u:
                                               pp * 16 + 2 * u + 2, msl],
                                    rhs=otg[:, pp, 2 * u:2 * u + 2, :],
                                    start=(pp == 0 and u == 0),
                                    stop=(pp == 1 and u == 7),
                                    perf_mode=DR,
                                )
                        mt = mc * 4 + m
                        nc.vector.tensor_scalar_mul(
                            cacc[:, mt, :], ps, 1.0 / WO_SCALE
                        )
                        nc.vector.tensor_scalar_mul(
                            cacc8[:, mt, :], ps, 1.0 / WO_SCALE
                        )

                # gate MLP, fully local
                g8 = p34.tile([128, GH // 128, BS], F8, tag="g8", bufs=1)
                for gm in range(GH // 128):
                    ps = psum.tile([128, BS], F32, tag="mm", bufs=4,
                                   name="ps_g1c")
                    for k in range(KP):
                        nc.tensor.matmul(
                            ps,
                            lhsT=gw1c_sb[:, 2 * k:2 * k + 2,
                                         gm * 128:(gm + 1) * 128],
                            rhs=cacc8[:, 2 * k:2 * k + 2, :],
                            start=(k == 0),
                            stop=(k == KP - 1),
                            perf_mode=DR,
                        )
                    gsum = p34.tile([128, BS], F32, tag="gsum", bufs=2)
                    nc.vector.tensor_add(gsum, ps, g1x_sb[:, gm, :])
                    nc.scalar.activation(g8[:, gm, :], gsum, GELU_FUNC,
                                         bias=gb1_sb[:, gm:gm + 1], scale=1.0)

                for m in range(KT_TILES):
                    msl = slice(m * 128, (m + 1) * 128)
                    ps = psum.tile([128, BS], F32, tag="mm", bufs=4,
                                   name="ps_gw2")
                    for k in range(GH // 256):
                        nc.tensor.matmul(
                            ps,
                            lhsT=gw2_sb[:, 2 * k:2 * k + 2, msl],
                            rhs=g8[:, 2 * k:2 * k + 2, :],
                            start=(k == 0),
                            stop=(k == GH // 256 - 1),
                            perf_mode=DR,
                        )
                    gate_ch = p34.tile([128, BS], BF16, tag="gate", bufs=2)
                    nc.scalar.activation(
                        gate_ch, ps,
                        mybir.ActivationFunctionType.Sigmoid,
                        bias=gb2_sb[:, m:m + 1], scale=1.0,
                    )
                    outt = p34.tile([128, BS], F32, tag="outt", bufs=2)
                    nc.vector.tensor_mul(outt, gate_ch, cacc[:, m, :])
                    nc.sync.dma_start(out=out_d[msl, :], in_=outt)

    nc.compile()
    return nc


def _make_in_maps(inputs):
    f32 = np.float32
    f8 = ml_dtypes.float8_e4m3
    X = np.asarray(inputs["hidden_states"], dtype=f32)
    mask = np.asarray(inputs["attention_mask"])
    Wq = np.asarray(inputs["Wq"], dtype=f32)
    Wk = np.asarray(inputs["Wk"], dtype=f32)
    Wv = np.asarray(inputs["Wv"], dtype=f32)
    Wo = np.asarray(inputs["Wo"], dtype=f32)
    gW1 = np.asarray(inputs["gW1"], dtype=f32)
    gb1 = np.asarray(inputs["gb1"], dtype=f32)
    gW2 = np.asarray(inputs["gW2"], dtype=f32)
    gb2 = np.asarray(inputs["gb2"], dtype=f32)

    XT8 = np.ascontiguousarray(X.T).astype(f8)           # [4096, 2048]
    # Wo row permutation to match the otg k-tile order (pp, r, hl2):
    # otg row (pp*2048 + r*256 + hl2*128 + d) holds global head
    # (4r + 2pp + hl2), dim d.
    perm = np.empty(HID, dtype=np.int64)
    for pp in range(2):
        for r in range(NC_):
            for hl2 in range(2):
                g = 4 * r + 2 * pp + hl2
                o = pp * 2048 + r * 256 + hl2 * 128
                perm[o:o + 128] = np.arange(g * 128, (g + 1) * 128)
    Wo_p = np.ascontiguousarray((Wo[perm] * WO_SCALE).astype(f8))
    maskb = (np.where(mask, 0.0, -1e30) - EXP_SHIFT).astype(f32)  # [2048]
    maskb_t = np.ascontiguousarray(maskb.reshape(JT, 128).T)  # [128, 16]
    diagm = np.ascontiguousarray(1.0 - np.eye(128, dtype=f32))

    gw1x8 = np.ascontiguousarray(gW1[:HID].astype(f8))
    gw1c8 = np.ascontiguousarray(gW1[HID:].astype(f8))
    gw28 = np.ascontiguousarray(gW2.astype(f8))
    gb1_t = np.ascontiguousarray(gb1.reshape(GH // 128, 128).T)
    gb2_t = np.ascontiguousarray(gb2.reshape(KT_TILES, 128).T)

    in_maps = []
    for c in range(NC_):
        hsl = slice(c * HS, (c + 1) * HS)
        in_maps.append({
            "xt8": XT8,
            "xo8": np.ascontiguousarray(XT8[:, c * BS:(c + 1) * BS]),
            "wq": np.ascontiguousarray(Wq[:, hsl].astype(f8)),
            "wk": np.ascontiguousarray(Wk[:, hsl].astype(f8)),
            "wv": np.ascontiguousarray(Wv[:, hsl].astype(f8)),
            "wo": Wo_p,
            "gw1x": gw1x8,
            "gw1c": gw1c8,
            "gw2": gw28,
            "gb1": gb1_t,
            "gb2": gb2_t,
            "maskb": maskb_t,
            "diagm": diagm,
        })
    return in_maps


_NC_CACHE = None


def _run(inputs, trace=False):
    global _NC_CACHE
    if _NC_CACHE is None:
        _NC_CACHE = _build_program()
    nc = _NC_CACHE
    in_maps = _make_in_maps(inputs)
    res = bass_utils.run_bass_kernel_spmd(
        nc, in_maps, core_ids=list(range(NC_)), trace=trace
    )
    shards = [np.asarray(res.results[c]["out"], dtype=np.float32)
              for c in range(NC_)]
    gated = np.concatenate(shards, axis=1).T  # gate * cross, [2048, 4096]
    out = np.asarray(inputs["hidden_states"], dtype=np.float32) + gated
    return np.ascontiguousarray(out), res


def kernel(**inputs) -> np.ndarray:
    out, _ = _run(inputs, trace=False)
    return out


# revision 11
# speedup vs baseline: 1.1447x; 1.1447x over previous
"""CrossBatchAttention Trainium2 kernel — 8-core tensor-parallel SPMD.

v4: head-parallel attention + BATCH-parallel out_proj/gate via AllToAll.

The expensive collectives of the tensor-parallel formulation (8MB OT
AllGather + gate ReduceScatter/AllGather chain, ~290us of CC) are replaced
by two small AllToAlls (heads 0-2 at the 3/4 mark of attention, head 3 at
the end): after attention, core c keeps its 4 heads' O^T for batch shard r
and sends [., 256] fp8 slices to core r. Each core then owns the FULL
attention output for its 256-row batch shard and runs out_proj + the
whole gate MLP locally with full (fp8) weights streamed into SBUF during
phase 2 (the sync-DMA queue is kept free of phase-2 traffic for this).

fp8 + MatmulPerfMode.DoubleRow everywhere the contraction depth allows.
Softmax: exp is shifted by a constant C (folded into the mask bias) so P
stays under fp8e4's 240 max; numerator and denominator share the same fp8
P so quantization largely cancels. The self-attention diagonal is zeroed
in PSUM before exp (leaves a negligible exp(-C) self-weight).

Per core c (of 8):
  phase 1: QT/KT bf16 [512,2048], V fp8 (4 local heads) from fp8 X^T and
           fp8 resident weights via DoubleRow, streamed in batch-quarters;
           g1x = gW1[:H]^T @ X^T[:, own 256 cols] (full gate hidden, own
           batch shard) from a per-core input slice.
  phase 2: per (head, batch-quarter): S^T = K^T@Q^T per j-tile (bf16),
           diag-zero in PSUM, Exp(scale*s + maskb - C) straight to fp8,
           denominator + O^T via DoubleRow over j-tile pairs, normalize,
           scatter O^T fp8 into A2A send buffers (gpsimd DMA queue).
  phase 3: otg [4096, 256] = gathered O^T for own shard; cross^T[:, own]
           = Wo_p^T @ otg (full hid, DoubleRow, Wo streamed in 8 chunks);
           g = gelu(g1x + gW1c^T @ cross8 + b1) fully local; gate =
           sigmoid(gW2^T g + b2); out^T = gate * cross^T  [4096, 256].
Host: concat 8 [4096,256] shards along batch, transpose, add X.
"""

import numpy as np
import ml_dtypes

import concourse.bass as bass
import concourse.mybir as mybir
import concourse.tile as tile
from concourse import bacc
from concourse import bass_utils

BF16 = mybir.dt.bfloat16
F32 = mybir.dt.float32
F8 = mybir.dt.float8e4
WO_SCALE = 64.0
EXP_SHIFT = 5.0          # exp(s - C): keeps P < 240 (fp8e4 max); cancels in softmax

B = 2048
HID = 4096
NH = 32
HD = 128
GH = 1024
NC_ = 8
HPC = NH // NC_          # heads per core = 4
HS = HID // NC_          # hid shard = 512 (attention head shard)
BS = B // NC_            # batch shard = 256 (out_proj/gate shard)
SCALE = 1.0 / float(np.sqrt(HD))

KT_TILES = HID // 128    # 32 k-tiles over the 4096 contraction
KP = KT_TILES // 2       # 16 DoubleRow k-pairs
JT = B // 128            # 16 j-tiles over keys
JP = JT // 2             # 8 DoubleRow j-pairs
IC = B // 512            # 4 i-chunks of 512 over batch (attention tiling)

DR = mybir.MatmulPerfMode.DoubleRow

GELU_FUNC = mybir.ActivationFunctionType.Gelu


def _build_program():
    nc = bacc.Bacc(
        "TRN2",
        target_bir_lowering=False,
        debug=False,
        enable_asserts=False,
        num_devices=NC_,
    )

    # ---- I/O declarations (per-core shapes) ----
    xt_d = nc.dram_tensor("xt8", [HID, B], F8, kind="ExternalInput").ap()
    xo_d = nc.dram_tensor("xo8", [HID, BS], F8, kind="ExternalInput").ap()
    wq_d = nc.dram_tensor("wq", [HID, HS], F8, kind="ExternalInput").ap()
    wk_d = nc.dram_tensor("wk", [HID, HS], F8, kind="ExternalInput").ap()
    wv_d = nc.dram_tensor("wv", [HID, HS], F8, kind="ExternalInput").ap()
    wo_d = nc.dram_tensor("wo", [HID, HID], F8, kind="ExternalInput").ap()
    gw1x_d = nc.dram_tensor("gw1x", [HID, GH], F8, kind="ExternalInput").ap()
    gw1c_d = nc.dram_tensor("gw1c", [HID, GH], F8, kind="ExternalInput").ap()
    gw2_d = nc.dram_tensor("gw2", [GH, HID], F8, kind="ExternalInput").ap()
    gb1_d = nc.dram_tensor("gb1", [128, GH // 128], F32,
                           kind="ExternalInput").ap()
    gb2_d = nc.dram_tensor("gb2", [128, KT_TILES], F32,
                           kind="ExternalInput").ap()
    maskb_d = nc.dram_tensor("maskb", [128, JT], F32, kind="ExternalInput").ap()
    diagm_d = nc.dram_tensor("diagm", [128, 128], F32, kind="ExternalInput").ap()
    out_d = nc.dram_tensor("out", [HID, BS], F32, kind="ExternalOutput").ap()

    groups = [list(range(NC_))]

    with tile.TileContext(nc) as tc:
        with (
            tc.tile_pool(name="persist", bufs=1) as persist,
            tc.tile_pool(name="psum", bufs=1, space="PSUM") as psum,
            tc.tile_pool(name="dram", bufs=1, space="DRAM") as dram,
        ):
            # ---------- persistent SBUF ----------
            g1x_sb = persist.tile([128, GH // 128, BS], F32)  # 1MB
            maskb_sb = persist.tile([128, JT], F32)
            diagm_sb = persist.tile([128, 128], F32)
            ones_sb = persist.tile([128, 2, 128], F8)
            gb1_sb = persist.tile([128, GH // 128], F32)
            gb2_sb = persist.tile([128, KT_TILES], F32)

            nc.gpsimd.dma_start(out=maskb_sb, in_=maskb_d)
            nc.gpsimd.dma_start(out=diagm_sb, in_=diagm_d)
            nc.gpsimd.dma_start(out=gb1_sb, in_=gb1_d)
            nc.gpsimd.dma_start(out=gb2_sb, in_=gb2_d)
            nc.vector.memset(ones_sb, 1.0)

            # ---------- DRAM buffers for the A2A ----------
            # src_a: heads 0-2 (block r = [384, 256]); src_b: head 3
            # (block r = [128, 256]).
            src_a = dram.tile([NC_ * 384, BS], F8, name="src_a")
            src_b = dram.tile([NC_ * 128, BS], F8, name="src_b")
            dst_a = dram.tile([NC_ * 384, BS], F8, name="dst_a")
            dst_b = dram.tile([NC_ * 128, BS], F8, name="dst_b")

            warm_i = dram.tile([NC_ * 16, 64], F8)
            warm_o = dram.tile([NC_ * 16, 64], F8)
            nc.gpsimd.collective_compute(
                "AllToAll", mybir.AluOpType.bypass, replica_groups=groups,
                ins=[warm_i.opt()], outs=[warm_o.opt()],
            )

            # =====================================================
            # Phase 1: projections, streamed in batch-quarters.
            # All weights fp8, resident; DoubleRow over k-pairs.
            # DMA issue order: wq, first xt quarter, then the rest.
            # =====================================================
            with tc.tile_pool(name="p12", bufs=1) as p12:
                qt_sb = p12.tile([128, HPC, B], BF16)     # [d, head, i] 2MB
                kt_sb = p12.tile([128, HPC, B], BF16)     # 2MB
                v_sb = p12.tile([128, JT, HS], F8)        # [j_in, jt, hd] 1MB

                with tc.tile_pool(name="p1", bufs=1) as p1:
                    wq_sb = p1.tile([128, KT_TILES, HS], F8, tag="wq", bufs=1)
                    wk_sb = p1.tile([128, KT_TILES, HS], F8, tag="wk", bufs=1)
                    wv_sb = p1.tile([128, KT_TILES, HS], F8, tag="wv", bufs=1)
                    gw1x_sb = p1.tile([128, KT_TILES, GH], F8, tag="gw1x",
                                      bufs=1)
                    xo_sb = p1.tile([128, KT_TILES, BS], F8, tag="xo", bufs=1)
                    xt_tiles = [
                        p1.tile([128, KT_TILES, 512], F8, tag="xt", bufs=2,
                                name=f"xt{q}")
                        for q in range(IC)
                    ]

                    def load_w(wd, wsb, n=4):
                        hh_w = 4096 // n
                        for hh in range(n):
                            nc.sync.dma_start(
                                out=wsb[:, hh * (KT_TILES // n):
                                        (hh + 1) * (KT_TILES // n), :],
                                in_=wd[hh * hh_w:(hh + 1) * hh_w, :].rearrange(
                                    "(t p) m -> p t m", p=128
                                ),
                            )

                    def load_xt(q):
                        isl = slice(q * 512, (q + 1) * 512)
                        for kk in range(4):
                            nc.sync.dma_start(
                                out=xt_tiles[q][:, kk * 8:(kk + 1) * 8, :],
                                in_=xt_d[kk * 1024:(kk + 1) * 1024,
                                         isl].rearrange(
                                    "(t p) i -> p t i", p=128
                                ),
                            )

                    load_w(wq_d, wq_sb)
                    load_xt(0)
                    load_w(wk_d, wk_sb)
                    load_xt(1)
                    load_w(wv_d, wv_sb)
                    load_w(gw1x_d, gw1x_sb)
                    nc.sync.dma_start(
                        out=xo_sb,
                        in_=xo_d.rearrange("(t p) m -> p t m", p=128),
                    )

                    for q in range(IC):  # 4 quarters of 512 batch elems
                        isl = slice(q * 512, (q + 1) * 512)
                        xt_q = xt_tiles[q]
                        if q + 2 < IC:
                            load_xt(q + 2)

                        for wsb, dst in ((wq_sb, qt_sb), (wk_sb, kt_sb)):
                            for m in range(4):
                                msl = slice(m * 128, (m + 1) * 128)
                                ps = psum.tile([128, 512], F32, tag="mm",
                                               bufs=4, name="ps_pr")
                                for k in range(KP):
                                    nc.tensor.matmul(
                                        ps,
                                        lhsT=wsb[:, 2 * k:2 * k + 2, msl],
                                        rhs=xt_q[:, 2 * k:2 * k + 2, :],
                                        start=(k == 0),
                                        stop=(k == KP - 1),
                                        perf_mode=DR,
                                    )
                                nc.vector.tensor_copy(dst[:, m, isl], ps)
                        # V in natural [j, d] layout: lhsT = X^T tiles
                        for it in range(4):
                            ps = psum.tile([128, 512], F32, tag="mm", bufs=4,
                                           name="ps_v")
                            for k in range(KP):
                                nc.tensor.matmul(
                                    ps,
                                    lhsT=xt_q[:, 2 * k:2 * k + 2,
                                              it * 128:(it + 1) * 128],
                                    rhs=wv_sb[:, 2 * k:2 * k + 2, :],
                                    start=(k == 0),
                                    stop=(k == KP - 1),
                                    perf_mode=DR,
                                )
                            nc.vector.tensor_copy(v_sb[:, q * 4 + it, :], ps)

                    # gate W1 X-part: full gate hidden, own batch shard
                    for gm in range(GH // 128):
                        ps = psum.tile([128, BS], F32, tag="mm", bufs=4,
                                       name="ps_g1x")
                        for k in range(KP):
                            nc.tensor.matmul(
                                ps,
                                lhsT=gw1x_sb[:, 2 * k:2 * k + 2,
                                             gm * 128:(gm + 1) * 128],
                                rhs=xo_sb[:, 2 * k:2 * k + 2, :],
                                start=(k == 0),
                                stop=(k == KP - 1),
                                perf_mode=DR,
                            )
                        nc.vector.tensor_copy(g1x_sb[:, gm, :], ps)

                # p1 (weights + xt) is closed here; p34's weight loads
                # below reuse its SBUF and stream during phase 2.
                with tc.tile_pool(name="p34", bufs=1) as p34:
                    gw1c_sb = p34.tile([128, KT_TILES, GH], F8, tag="gw1c",
                                       bufs=1)
                    for hh in range(4):
                        nc.sync.dma_start(
                            out=gw1c_sb[:, hh * 8:(hh + 1) * 8, :],
                            in_=gw1c_d[hh * 1024:(hh + 1) * 1024,
                                       :].rearrange("(t p) m -> p t m",
                                                    p=128),
                        )
                    wo_tiles = []
                    for mc in range(8):
                        wo_mc = p34.tile([128, KT_TILES, 512], F8, tag="wo",
                                         bufs=3, name=f"wo{mc}")
                        for hh in range(4):
                            nc.sync.dma_start(
                                out=wo_mc[:, hh * 8:(hh + 1) * 8, :],
                                in_=wo_d[hh * 1024:(hh + 1) * 1024,
                                         mc * 512:(mc + 1) * 512].rearrange(
                                    "(t p) m -> p t m", p=128
                                ),
                            )
                        wo_tiles.append(wo_mc)

                    # =====================================================
                    # Phase 2: attention per (head, batch-quarter).
                    # otc scatter on the gpsimd queue (sync queue stays
                    # free for the weight streaming above).
                    # =====================================================
                    with tc.tile_pool(name="p2", bufs=1) as p2:
                        for h in range(HPC):
                            for q in range(IC):
                                qsl = slice(q * 512, (q + 1) * 512)
                                den_ps = psum.tile([128, 512], F32,
                                                   tag="den", bufs=2)
                                ot_ps = psum.tile([128, 512], F32, tag="ot",
                                                  bufs=2)
                                pt = p2.tile([128, JT, 512], F8, tag="pt",
                                             bufs=2)
                                for j in range(JT):
                                    st = psum.tile([128, 512], F32, tag="mm",
                                                   bufs=4, name="st")
                                    nc.tensor.matmul(
                                        st,
                                        lhsT=kt_sb[:, h,
                                                   j * 128:(j + 1) * 128],
                                        rhs=qt_sb[:, h, qsl],
                                        start=True,
                                        stop=True,
                                    )
                                    if j // 4 == q:
                                        c0 = (j % 4) * 128
                                        nc.vector.tensor_mul(
                                            st[:, c0:c0 + 128],
                                            st[:, c0:c0 + 128],
                                            diagm_sb,
                                        )
                                    nc.scalar.activation(
                                        pt[:, j, :],
                                        st,
                                        mybir.ActivationFunctionType.Exp,
                                        bias=maskb_sb[:, j:j + 1],
                                        scale=SCALE,
                                    )
                                for j in range(JP):
                                    nc.tensor.matmul(
                                        den_ps,
                                        lhsT=ones_sb,
                                        rhs=pt[:, 2 * j:2 * j + 2, :],
                                        start=(j == 0),
                                        stop=(j == JP - 1),
                                        perf_mode=DR,
                                    )
                                    nc.tensor.matmul(
                                        ot_ps,
                                        lhsT=v_sb[:, 2 * j:2 * j + 2,
                                                  h * 128:(h + 1) * 128],
                                        rhs=pt[:, 2 * j:2 * j + 2, :],
                                        start=(j == 0),
                                        stop=(j == JP - 1),
                                        perf_mode=DR,
                                    )
                                rec = p2.tile([128, 512], F32, tag="rec",
                                              bufs=2)
                                nc.vector.reciprocal_approx_fast(
                                    out=rec, in_=den_ps)
                                otc = p2.tile([128, 512], F8, tag="otc",
                                              bufs=2)
                                nc.vector.tensor_mul(otc, ot_ps, rec)
                                # scatter into A2A send buffers
                                for half in range(2):
                                    r = 2 * q + half
                                    if h < 3:
                                        dst = src_a[r * 384 + h * 128:
                                                    r * 384 + h * 128 + 128,
                                                    :]
                                    else:
                                        dst = src_b[r * 128:r * 128 + 128, :]
                                    nc.gpsimd.dma_start(
                                        out=dst,
                                        in_=otc[:, half * 256:
                                                half * 256 + 256],
                                    )
                            if h == 2:
                                nc.gpsimd.collective_compute(
                                    "AllToAll", mybir.AluOpType.bypass,
                                    replica_groups=groups,
                                    ins=[src_a.opt()], outs=[dst_a.opt()],
                                )
                            elif h == 3:
                                nc.gpsimd.collective_compute(
                                    "AllToAll", mybir.AluOpType.bypass,
                                    replica_groups=groups,
                                    ins=[src_b.opt()], outs=[dst_b.opt()],
                                )

                    # =====================================================
                    # Phase 3: out_proj + gate MLP, fully local on the
                    # own 256-row batch shard.
                    # =====================================================
                    gw2_sb = p34.tile([128, GH // 128, HID], F8, tag="gw2",
                                      bufs=1)
                    for hh in range(2):
                        nc.sync.dma_start(
                            out=gw2_sb[:, hh * 4:(hh + 1) * 4, :],
                            in_=gw2_d[hh * 512:(hh + 1) * 512, :].rearrange(
                                "(t p) m -> p t m", p=128
                            ),
                        )
                    cacc8 = p34.tile([128, KT_TILES, BS], F8, tag="cacc8",
                                     bufs=1)
                    # gathered O^T [128, (r, hl), 256]; hl 0-2 from dst_a,
                    # hl 3 from dst_b
                    otg = p34.tile([128, NC_, 4, BS], F8, tag="otg", bufs=1)
                    for hl in range(3):
                        nc.scalar.dma_start(
                            out=otg[:, :, hl, :],
                            in_=dst_a.rearrange("(r t p) i -> p r t i",
                                                p=128, t=3)[:, :, hl, :],
                        )
                    nc.scalar.dma_start(
                        out=otg[:, :, 3, :],
                        in_=dst_b.rearrange("(r p) i -> p r i", p=128),
                    )

                    # out_proj: cross^T[:, own shard]
                    for mc in range(8):
                        wo_mc = wo_tiles[mc]
                        for m in range(4):
                            ps = psum.tile([128, BS], F32, tag="mm", bufs=4,
                                           name="ps_wo")
                            for u in range(KP):
                                nc.tensor.matmul(
                                    ps,
                                    lhsT=wo_mc[:, 2 * u:2 * u + 2,
                                               m * 128:(m + 1) * 128],
                                    rhs=otg[:, u // 2, 2 * (u % 2):
                                            2 * (u % 2) + 2, :],
                                    start=(u == 0),
                                    stop=(u == KP - 1),
                                    perf_mode=DR,
                                )
                            mt = mc * 4 + m
                            nc.vector.tensor_scalar_mul(
                                cacc8[:, mt, :], ps, 1.0 / WO_SCALE
                            )

                    # gate MLP, fully local
                    g8 = p34.tile([128, GH // 128, BS], F8, tag="g8", bufs=1)
                    for gm in range(GH // 128):
                        ps = psum.tile([128, BS], F32, tag="mm", bufs=4,
                                       name="ps_g1c")
                        for k in range(KP):
                            nc.tensor.matmul(
                                ps,
                                lhsT=gw1c_sb[:, 2 * k:2 * k + 2,
                                             gm * 128:(gm + 1) * 128],
                                rhs=cacc8[:, 2 * k:2 * k + 2, :],
                                start=(k == 0),
                                stop=(k == KP - 1),
                                perf_mode=DR,
                            )
                        gsum = p34.tile([128, BS], F32, tag="gsum", bufs=3)
                        nc.vector.tensor_add(gsum, ps, g1x_sb[:, gm, :])
                        nc.scalar.activation(g8[:, gm, :], gsum, GELU_FUNC,
                                             bias=gb1_sb[:, gm:gm + 1],
                                             scale=1.0)

                    for m in range(KT_TILES):
                        msl = slice(m * 128, (m + 1) * 128)
                        ps = psum.tile([128, BS], F32, tag="mm", bufs=4,
                                       name="ps_gw2")
                        for k in range(GH // 256):
                            nc.tensor.matmul(
                                ps,
                                lhsT=gw2_sb[:, 2 * k:2 * k + 2, msl],
                                rhs=g8[:, 2 * k:2 * k + 2, :],
                                start=(k == 0),
                                stop=(k == GH // 256 - 1),
                                perf_mode=DR,
                            )
                        gate_ch = p34.tile([128, BS], BF16, tag="gate",
                                           bufs=3)
                        nc.scalar.activation(
                            gate_ch, ps,
                            mybir.ActivationFunctionType.Sigmoid,
                            bias=gb2_sb[:, m:m + 1], scale=1.0,
                        )
                        outt = p34.tile([128, BS], F32, tag="outt", bufs=3)
                        nc.vector.tensor_mul(outt, gate_ch, cacc8[:, m, :])
                        nc.sync.dma_start(out=out_d[msl, :], in_=outt)

    nc.compile()
    return nc


def _make_in_maps(inputs):
    f32 = np.float32
    f8 = ml_dtypes.float8_e4m3
    X = np.asarray(inputs["hidden_states"], dtype=f32)
    mask = np.asarray(inputs["attention_mask"])
    Wq = np.asarray(inputs["Wq"], dtype=f32)
    Wk = np.asarray(inputs["Wk"], dtype=f32)
    Wv = np.asarray(inputs["Wv"], dtype=f32)
    Wo = np.asarray(inputs["Wo"], dtype=f32)
    gW1 = np.asarray(inputs["gW1"], dtype=f32)
    gb1 = np.asarray(inputs["gb1"], dtype=f32)
    gW2 = np.asarray(inputs["gW2"], dtype=f32)
    gb2 = np.asarray(inputs["gb2"], dtype=f32)

    XT8 = np.ascontiguousarray(X.T).astype(f8)           # [4096, 2048]
    # Wo row permutation to match the otg k-tile order (r, hl):
    # otg row ((4r + hl)*128 + d) holds global head (4r + hl), dim d —
    # identity here, but keep the scaffold for layout changes.
    Wo_p = np.ascontiguousarray((Wo * WO_SCALE).astype(f8))
    maskb = (np.where(mask, 0.0, -1e30) - EXP_SHIFT).astype(f32)  # [2048]
    maskb_t = np.ascontiguousarray(maskb.reshape(JT, 128).T)  # [128, 16]
    diagm = np.ascontiguousarray(1.0 - np.eye(128, dtype=f32))

    gw1x8 = np.ascontiguousarray(gW1[:HID].astype(f8))
    gw1c8 = np.ascontiguousarray(gW1[HID:].astype(f8))
    gw28 = np.ascontiguousarray(gW2.astype(f8))
    gb1_t = np.ascontiguousarray(gb1.reshape(GH // 128, 128).T)
    gb2_t = np.ascontiguousarray(gb2.reshape(KT_TILES, 128).T)

    in_maps = []
    for c in range(NC_):
        hsl = slice(c * HS, (c + 1) * HS)
        in_maps.append({
            "xt8": XT8,
            "xo8": np.ascontiguousarray(XT8[:, c * BS:(c + 1) * BS]),
            "wq": np.ascontiguousarray(Wq[:, hsl].astype(f8)),
            "wk": np.ascontiguousarray(Wk[:, hsl].astype(f8)),
            "wv": np.ascontiguousarray(Wv[:, hsl].astype(f8)),
            "wo": Wo_p,
            "gw1x": gw1x8,
            "gw1c": gw1c8,
            "gw2": gw28,
            "gb1": gb1_t,
            "gb2": gb2_t,
            "maskb": maskb_t,
            "diagm": diagm,
        })
    return in_maps


_NC_CACHE = None


def _run(inputs, trace=False):
    global _NC_CACHE
    if _NC_CACHE is None:
        _NC_CACHE = _build_program()
    nc = _NC_CACHE
    in_maps = _make_in_maps(inputs)
    res = bass_utils.run_bass_kernel_spmd(
        nc, in_maps, core_ids=list(range(NC_)), trace=trace
    )
    shards = [np.asarray(res.results[c]["out"], dtype=np.float32)
              for c in range(NC_)]
    gated = np.concatenate(shards, axis=1).T  # gate * cross, [2048, 4096]
    out = np.asarray(inputs["hidden_states"], dtype=np.float32) + gated
    return np.ascontiguousarray(out), res


def kernel(**inputs) -> np.ndarray:
    out, _ = _run(inputs, trace=False)
    return out
